# revision 59
# baseline (speedup 1.0000x reference)
"""Pairwise squared Euclidean distance on Trainium2, sharded over 8 NeuronCores.

dist[i, j] = ||s_i - t_j||^2 = s_sq[i] + t_sq[j] - 2 * (s @ t.T)[i, j]

Sharding: rows of s (and of the output) are split across the 8 cores;
t is replicated to every core. Each core computes a [2048, 16384] tile.

Per-core device program (single-matmul bf16 path):
  The tolerance (rel 2e-2) allows computing the cross term from the bf16
  hi parts alone: with A = bf16(-2*s)^T and B = bf16(t)^T, a single K=65
  matmul per output tile produces  -2*s@t.T + t_sq  in fp32 PSUM (row 64
  of A is all-ones against row 64 of B holding bf16(t_sq), computed on PE
  from an all-ones stationary operand over square(B)).  Measured rel err
  of this scheme is ~2e-3.  The exact fp32 per-partition s_sq[i] (DVE
  square + reduce) is added during the PSUM->SBUF copy (one fused
  [128, 1024] ACT-bias / DVE-tensor_scalar op per two PSUM banks), and
  staging tiles are DMA'd to the output on alternating HWDGE rings
  (SP / ACT).

  DMA traffic is minimized: both inputs are fetched with ONE strided DMA
  per 2048-row chunk in a 16-rows-per-partition grouped layout whose
  innermost contiguous run is 4KB (full DMA bus rate; <512B runs pay a
  2x penalty), then PE-transposed [128, 64] a column-view at a time;
  each transpose group lands in A/B via an affine stride-16 free-dim
  scatter that reconstructs global column order.  For s this means tile
  m covers rows {16p + m}, which the output DMA addresses with an affine
  partition stride of 16 rows -- same descriptor efficiency as row-major.
  t_sq is computed from each [64, 512] PSUM transpose group directly
  (square -> all-ones matmul -> row-64 scatter), so it never waits on a
  whole-chunk barrier.

  The main loop is emitted group-by-group (one 2048-column group per
  chunk); ALL chunk loads are queued upfront (~13us of input transfers
  exactly fill the DMA-idle prep window) and chunk g+1's
  engine processing is emitted after group g's tiles, so the first
  output DMA is gated only on chunk 0's prep while Tile's range-accurate
  dependency tracking overlaps later prep with the saturated output
  stream.  The output is write-only traffic of 134 MB/core; the DMA
  model's bus rate (360 GB/s) puts the floor at ~373us, and the kernel
  sustains ~392us total (DMA busy ~386us, ~98.4% occupancy; remaining
  idle is the fixed start/tail latency plus a ~2.7us production ramp).
"""

import numpy as np

import concourse.mybir as mybir
import concourse.tile as tile
from concourse import bacc
from concourse.masks import make_identity

F32 = mybir.dt.float32
BF16 = mybir.dt.bfloat16

N_CORES = 8
N, Q, D = 16384, 16384, 64
N_SHARD = N // N_CORES  # 2048


def build_nc(n_rows=N_SHARD, q=Q, d=D, chunk=2048):
    assert n_rows % 128 == 0 and q % chunk == 0
    assert chunk % 512 == 0 and d == 64
    m_tiles = n_rows // 128          # 16
    n_chunks = q // chunk            # 8
    t_per_chunk = chunk // 128       # 16
    K = d + 1                        # 65: d rows of sh, row 64 = ones / t_sq

    nc = bacc.Bacc()
    s = nc.dram_tensor("s", [n_rows, d], F32, kind="ExternalInput")
    t = nc.dram_tensor("t", [q, d], F32, kind="ExternalInput")
    o = nc.dram_tensor("o", [n_rows, q], F32, kind="ExternalOutput")

    with tile.TileContext(nc) as tc:
        with (
            tc.tile_pool(name="const", bufs=1) as const,
            tc.tile_pool(name="work", bufs=4) as work,
            tc.tile_pool(name="chunks", bufs=4) as chunks,
            tc.tile_pool(name="stage", bufs=4) as stage,
            tc.tile_pool(name="psum_prep", bufs=3, space="PSUM") as psum_prep,
            tc.tile_pool(name="psum_pts", bufs=1, space="PSUM") as psum_pts,
            tc.tile_pool(name="psum_mm", bufs=2, space="PSUM") as psum_mm,
        ):
            S = const.tile([128, m_tiles * d], F32, name="S")

            identity = const.tile([128, 128], F32, name="identity")
            make_identity(nc, identity)
            neg2I = const.tile([128, 128], F32, name="neg2I")
            make_identity(nc, neg2I)
            nc.scalar.mul(neg2I, neg2I, -2.0)
            ones_mat = const.tile([d, 128], BF16, name="ones_mat")
            nc.vector.memset(ones_mat, 1.0)

            # PE warmup: dense fp32 matmuls to trip the HAM clock gate from
            # 4/8 (1.2 GHz) to 8/8 (2.4 GHz) early. The tiny DMA (on the
            # ACT ring, so the SP ring's input loads are not stalled) keeps
            # the chain live through DCE; the real output of that region is
            # written later (WAW-ordered).
            pw = psum_prep.tile([128, 128], F32, name="pw", tag="pp")
            for _ in range(7):
                nc.tensor.matmul(pw, identity, identity, start=True, stop=True)
            warm_sb = const.tile([1, 1], F32, name="warm_sb")
            nc.scalar.copy(warm_sb, pw[0:1, 0:1])
            nc.scalar.dma_start(out=o[0:1, 0:1], in_=warm_sb)

            A = const.tile([K, n_rows], BF16, name="A")   # sh rows 0..63, 64=ones
            B = const.tile([K, q], BF16, name="B")        # th rows 0..63, 64=t_sq
            s_sq = const.tile([128, m_tiles], F32, name="s_sq")
            # single-partition memset is slow on DVE (1 lane); Pool runs
            # Memset at full efficiency and is otherwise idle
            nc.gpsimd.memset(A[64:65, :], 1.0)

            # ---- s prep: per-tile transpose (PE) + fused square-reduce.
            # 4 transposes share one [64, 512] PSUM tile so the bf16
            # conversion is 1 big copy instead of 4 small ones. ----
            def prep_A():
                for m4 in range(m_tiles // 4):
                    pss = psum_prep.tile([d, 512], F32, name="pss", tag="pp")
                    for k in range(4):
                        m = m4 * 4 + k
                        V = S[:, m * d : (m + 1) * d]
                        # window k of pss = V.T @ (-2 I) = -2 s^T (exact)
                        nc.tensor.matmul(
                            pss[:, k * 128 : (k + 1) * 128], V, neg2I,
                            start=True, stop=True,
                        )
                    dst = A[0:d, m4 * 512 : (m4 + 1) * 512]
                    if m4 % 2 == 0:
                        nc.scalar.copy(dst, pss)
                    else:
                        nc.vector.tensor_copy(dst, pss)

            def prep_ssq(m):
                # exact fp32 row sums of s^2 (native DVE ops -- the fused
                # tensor_tensor_reduce is custom-ucode and not loadable in
                # this runtime).  Emitted one tile ahead of its consumer in
                # group 0 so the 32 ops never pool up in front of the adds.
                V = S[:, m * d : (m + 1) * d]
                sqs = work.tile([128, d], F32, name="sqs", tag="sqs")
                nc.vector.tensor_mul(sqs, V, V)
                nc.vector.tensor_reduce(
                    s_sq[:, m : m + 1], sqs, mybir.AxisListType.X,
                    mybir.AluOpType.add,
                )

            # ---- t prep: the load and the engine processing are emitted
            # separately so loads can be queued far ahead ----
            g_tiles = {}

            def load_chunk(ch):
                base = ch * chunk
                # grouped layout: partition p holds t rows base+16p..+15,
                # giving 4KB contiguous runs (full DMA bus rate)
                G = chunks.tile(
                    [128, t_per_chunk * d], F32, name="G", tag="G", bufs=8
                )
                nc.sync.dma_start(
                    out=G[:, :].rearrange("p (j d) -> p j d", j=t_per_chunk, d=d),
                    in_=t[base : base + chunk, :].rearrange(
                        "(p j) d -> p j d", p=128, j=t_per_chunk
                    ),
                )
                g_tiles[ch] = G

            def transpose_chunk(ch):
                base = ch * chunk
                G = g_tiles.pop(ch)
                # B columns c = 16p + j: transpose view j, scatter stride 16.
                # All 16 transposes (3-deep PSUM rotation), with the
                # scatter-copy and a square of the just-written B columns
                # (in scatter order, so each square depends only on its own
                # quarter) interleaved on alternating engines.
                Bv = B[0:d, base : base + chunk].rearrange(
                    "e (p j) -> e j p", p=128, j=t_per_chunk
                )
                sqs4 = []
                for j4 in range(t_per_chunk // 4):
                    pst = psum_prep.tile([d, 512], F32, name="pst", tag="pp")
                    for k in range(4):
                        V = G[:, (j4 * 4 + k) * d : (j4 * 4 + k + 1) * d]
                        nc.tensor.transpose(
                            pst[:, k * 128 : (k + 1) * 128], V, identity
                        )
                    dst = Bv[:, j4 * 4 : (j4 + 1) * 4, :]
                    src = pst[:, :].rearrange("e (k p) -> e k p", k=4, p=128)
                    sqv = chunks.tile([d, 512], BF16, name="sq", tag="sq", bufs=4)
                    sqv3 = sqv[:, :].rearrange("e (k p) -> e k p", k=4, p=128)
                    if j4 % 2 == 0:
                        nc.scalar.copy(dst, src)
                        nc.vector.tensor_mul(sqv3, dst, dst)
                    else:
                        nc.vector.tensor_copy(dst, src)
                        nc.scalar.square(sqv3, dst)
                    sqs4.append(sqv)
                return sqs4

            def tsq_chunk(ch, sqs4):
                base = ch * chunk
                # t_sq = ones^T @ sq (bf16, 1 cycle/row), row 64 scattered
                # back with the same (k, p) pattern
                B64v = B[64:65, base : base + chunk].rearrange(
                    "e (p j) -> e j p", p=128, j=t_per_chunk
                )
                for j4, sqv in enumerate(sqs4):
                    pts = psum_pts.tile([128, 512], F32, name="pts", tag="pts")
                    nc.tensor.matmul(pts, ones_mat, sqv, start=True, stop=True)
                    tdst = B64v[:, j4 * 4 : (j4 + 1) * 4, :]
                    tsrc = pts[64:65, :].rearrange("e (k p) -> e k p", k=4, p=128)
                    if j4 % 2 == 0:
                        nc.scalar.copy(tdst, tsrc)
                    else:
                        nc.vector.tensor_copy(tdst, tsrc)

            def process_chunk(ch):
                tsq_chunk(ch, transpose_chunk(ch))

            # ---- main loop tile: one [128, chunk] staging tile.  PSUM
            # tiles span 2 banks (2 matmul windows) so each PSUM->SBUF
            # copy+bias is one [128, 1024] op; the two per tile alternate
            # ACT / DVE. ----
            # output rows of tile m are {16p + m}: affine partition stride
            ov = o[:, :].rearrange("(p m) q -> p m q", p=128, m=m_tiles)

            def main_tile(g, m, split_out=False):
                rows = slice(m * 128, (m + 1) * 128)
                stg = stage.tile([128, chunk], F32, name="stg", tag="stg")
                for ci in range(chunk // 1024):
                    ps = psum_mm.tile([128, 1024], F32, name="ps", tag="mm")
                    for k in range(2):
                        c = (g * chunk) // 512 + ci * 2 + k
                        cols = slice(c * 512, (c + 1) * 512)
                        nc.tensor.matmul(
                            ps[:, k * 512 : (k + 1) * 512], A[:, rows],
                            B[:, cols], start=True, stop=True,
                        )
                    dst = stg[:, ci * 1024 : (ci + 1) * 1024]
                    if (2 * m + ci) % 2 == 0:
                        nc.scalar.add(dst, ps, s_sq[:, m : m + 1])
                    else:
                        nc.vector.tensor_scalar_add(dst, ps, s_sq[:, m : m + 1])
                # alternate the two HWDGE rings (SP / ACT) for 2x the
                # DMA packet-processing throughput on the output stream
                out_eng = nc.sync if m % 2 == 0 else nc.scalar
                if not split_out:
                    out_eng.dma_start(
                        out=ov[:, m, g * chunk : (g + 1) * chunk], in_=stg
                    )
                else:
                    # last tile of the kernel: drain in two halves on both
                    # rings so the tail transfer is half as long
                    h = chunk // 2
                    nc.sync.dma_start(
                        out=ov[:, m, g * chunk : g * chunk + h], in_=stg[:, 0:h]
                    )
                    nc.scalar.dma_start(
                        out=ov[:, m, g * chunk + h : (g + 1) * chunk],
                        in_=stg[:, h:chunk],
                    )

            # software pipeline: the first 4 chunk loads are queued upfront
            # (G has 4 bufs); group g uses chunk g, whose engine processing
            # is emitted right after group g-1's tiles.  Only chunk 0 and
            # the s prep gate the first output DMA.
            # chunk 0 loads first so PE can transpose right after warmup;
            # s (needed later, by the first matmul/adds) follows it.  s uses
            # the same 16-rows-per-partition grouped layout (4KB runs):
            # partition p holds rows 16p..16p+15, so tile m covers rows
            # {16p + m}, and the output DMA addresses them with an affine
            # partition stride of 16 rows.  ALL chunk loads are queued
            # upfront (G has 8 bufs): their ~13us of transfers exactly fill
            # the DMA-idle prep window, so the saturated output stream
            # later never shares the bus with input traffic.
            load_chunk(0)
            nc.sync.dma_start(
                out=S[:, :].rearrange("p (m d) -> p m d", m=m_tiles, d=d),
                in_=s[:, :].rearrange("(p m) d -> p m d", p=128, m=m_tiles),
            )
            for ch in range(1, n_chunks):
                load_chunk(ch)
            # chunk 0: s-prep slots between the transposes and the t_sq
            # matmuls, filling the PE stall while the squares land
            _sq0 = transpose_chunk(0)
            prep_A()
            tsq_chunk(0, _sq0)
            prep_ssq(0)
            prep_ssq(1)
            for g in range(n_chunks):
                for m in range(m_tiles):
                    if g == 0 and m + 2 < m_tiles:
                        prep_ssq(m + 2)
                    # split the first tiles (smoother DMA ramp-up: each
                    # fused add releases its half immediately) and the very
                    # last one (halves the tail transfer)
                    split = (g == 0 and m < 2) or (
                        g == n_chunks - 1 and m == m_tiles - 1
                    )
                    main_tile(g, m, split_out=split)
                if g + 1 < n_chunks:
                    process_chunk(g + 1)

    nc.finalize()
    return nc


_NC_CACHE = {}


def _get_nc(key=None):
    if key is None:
        key = (N_SHARD, Q, D)
    if key not in _NC_CACHE:
        _NC_CACHE[key] = build_nc(*key)
    return _NC_CACHE[key]


def make_in_maps(inputs):
    s = np.asarray(inputs["s"], dtype=np.float32)
    t = np.asarray(inputs["t"], dtype=np.float32)
    assert s.shape == (N, D) and t.shape == (Q, D), (s.shape, t.shape)
    return [{"s": s[c * N_SHARD : (c + 1) * N_SHARD], "t": t} for c in range(N_CORES)]


def _run(inputs, **spmd_kwargs):
    from concourse.bass_utils import run_bass_kernel_spmd

    nc = _get_nc()
    in_maps = make_in_maps(inputs)
    res = run_bass_kernel_spmd(nc, in_maps, list(range(N_CORES)), **spmd_kwargs)
    out = np.concatenate([res.results[c]["o"] for c in range(N_CORES)], axis=0)
    return out, res


def kernel(**inputs):
    out, _ = _run(inputs)
    return out


# revision 68
# speedup vs baseline: 1.0031x; 1.0031x over previous
"""Pairwise squared Euclidean distance on Trainium2, sharded over 8 NeuronCores.

dist[i, j] = ||s_i - t_j||^2 = s_sq[i] + t_sq[j] - 2 * (s @ t.T)[i, j]

Sharding: rows of s (and of the output) are split across the 8 cores;
t is replicated to every core. Each core computes a [2048, 16384] tile.

Per-core device program (single-matmul bf16 path):
  The tolerance (rel 2e-2) allows computing the cross term from the bf16
  hi parts alone: with A = bf16(-2*s)^T and B = bf16(t)^T, a single K=65
  matmul per output tile produces  -2*s@t.T + t_sq  in fp32 PSUM (row 64
  of A is all-ones against row 64 of B holding bf16(t_sq), computed on PE
  from an all-ones stationary operand over square(B)).  Measured rel err
  of this scheme is ~2e-3.  The exact fp32 per-partition s_sq[i] (DVE
  square + reduce) is added during the PSUM->SBUF copy (one fused
  [128, 1024] ACT-bias / DVE-tensor_scalar op per two PSUM banks), and
  staging tiles are DMA'd to the output on alternating HWDGE rings
  (SP / ACT).

  DMA traffic is minimized: both inputs are fetched with ONE strided DMA
  per 2048-row chunk in a 16-rows-per-partition grouped layout whose
  innermost contiguous run is 4KB (full DMA bus rate; <512B runs pay a
  2x penalty), then PE-transposed [128, 64] a column-view at a time;
  each transpose group lands in A/B via an affine stride-16 free-dim
  scatter that reconstructs global column order.  For s this means tile
  m covers rows {16p + m}, which the output DMA addresses with an affine
  partition stride of 16 rows -- same descriptor efficiency as row-major.
  t_sq is computed from each [64, 512] PSUM transpose group directly
  (square -> all-ones matmul -> row-64 scatter), so it never waits on a
  whole-chunk barrier.

  The main loop is emitted group-by-group (one 2048-column group per
  chunk); ALL chunk loads are queued upfront (~13us of input transfers
  exactly fill the DMA-idle prep window) and chunk g+1's
  engine processing is emitted after group g's tiles, so the first
  output DMA is gated only on chunk 0's prep while Tile's range-accurate
  dependency tracking overlaps later prep with the saturated output
  stream.  The output is write-only traffic of 134 MB/core; the DMA
  model's bus rate (360 GB/s) puts the floor at ~373us, and the kernel
  sustains ~391us total (DMA busy ~386us, ~98.7% occupancy; remaining
  idle is the fixed start/tail latency plus a ~1.4us production ramp).
  PSUM banks: 2 transpose-rotation + 2 t_sq (double-buffered so the
  ones-matmuls overlap the row-64 scatter copies) + 2x2 main-loop.
"""

import numpy as np

import concourse.mybir as mybir
import concourse.tile as tile
from concourse import bacc
from concourse.masks import make_identity

F32 = mybir.dt.float32
BF16 = mybir.dt.bfloat16

N_CORES = 8
N, Q, D = 16384, 16384, 64
N_SHARD = N // N_CORES  # 2048


def build_nc(n_rows=N_SHARD, q=Q, d=D, chunk=2048):
    assert n_rows % 128 == 0 and q % chunk == 0
    assert chunk % 512 == 0 and d == 64
    m_tiles = n_rows // 128          # 16
    n_chunks = q // chunk            # 8
    t_per_chunk = chunk // 128       # 16
    K = d + 1                        # 65: d rows of sh, row 64 = ones / t_sq

    nc = bacc.Bacc()
    s = nc.dram_tensor("s", [n_rows, d], F32, kind="ExternalInput")
    t = nc.dram_tensor("t", [q, d], F32, kind="ExternalInput")
    o = nc.dram_tensor("o", [n_rows, q], F32, kind="ExternalOutput")

    with tile.TileContext(nc) as tc:
        with (
            tc.tile_pool(name="const", bufs=1) as const,
            tc.tile_pool(name="work", bufs=4) as work,
            tc.tile_pool(name="chunks", bufs=4) as chunks,
            tc.tile_pool(name="stage", bufs=4) as stage,
            tc.tile_pool(name="psum_prep", bufs=2, space="PSUM") as psum_prep,
            tc.tile_pool(name="psum_pts", bufs=2, space="PSUM") as psum_pts,
            tc.tile_pool(name="psum_mm", bufs=2, space="PSUM") as psum_mm,
        ):
            S = const.tile([128, m_tiles * d], F32, name="S")

            identity = const.tile([128, 128], F32, name="identity")
            make_identity(nc, identity)
            neg2I = const.tile([128, 128], F32, name="neg2I")
            make_identity(nc, neg2I)
            nc.scalar.mul(neg2I, neg2I, -2.0)
            ones_mat = const.tile([d, 128], BF16, name="ones_mat")
            nc.vector.memset(ones_mat, 1.0)

            # PE warmup: dense fp32 matmuls to trip the HAM clock gate from
            # 4/8 (1.2 GHz) to 8/8 (2.4 GHz) early. The tiny DMA (on the
            # ACT ring, so the SP ring's input loads are not stalled) keeps
            # the chain live through DCE; the real output of that region is
            # written later (WAW-ordered).
            pw = psum_prep.tile([128, 128], F32, name="pw", tag="pp")
            for _ in range(7):
                nc.tensor.matmul(pw, identity, identity, start=True, stop=True)
            warm_sb = const.tile([1, 1], F32, name="warm_sb")
            nc.scalar.copy(warm_sb, pw[0:1, 0:1])
            nc.scalar.dma_start(out=o[0:1, 0:1], in_=warm_sb)

            A = const.tile([K, n_rows], BF16, name="A")   # sh rows 0..63, 64=ones
            B = const.tile([K, q], BF16, name="B")        # th rows 0..63, 64=t_sq
            s_sq = const.tile([128, m_tiles], F32, name="s_sq")
            # single-partition memset is slow on DVE (1 lane); Pool runs
            # Memset at full efficiency and is otherwise idle
            nc.gpsimd.memset(A[64:65, :], 1.0)

            # ---- s prep: per-tile transpose (PE) + fused square-reduce.
            # 4 transposes share one [64, 512] PSUM tile so the bf16
            # conversion is 1 big copy instead of 4 small ones. ----
            def prep_A():
                for m4 in range(m_tiles // 4):
                    pss = psum_prep.tile([d, 512], F32, name="pss", tag="pp")
                    for k in range(4):
                        m = m4 * 4 + k
                        V = S[:, m * d : (m + 1) * d]
                        # window k of pss = V.T @ (-2 I) = -2 s^T (exact)
                        nc.tensor.matmul(
                            pss[:, k * 128 : (k + 1) * 128], V, neg2I,
                            start=True, stop=True,
                        )
                    dst = A[0:d, m4 * 512 : (m4 + 1) * 512]
                    if m4 % 2 == 0:
                        nc.scalar.copy(dst, pss)
                    else:
                        nc.vector.tensor_copy(dst, pss)

            def prep_ssq(m):
                # exact fp32 row sums of s^2 (native DVE ops -- the fused
                # tensor_tensor_reduce is custom-ucode and not loadable in
                # this runtime).  Emitted one tile ahead of its consumer in
                # group 0 so the 32 ops never pool up in front of the adds.
                V = S[:, m * d : (m + 1) * d]
                sqs = work.tile([128, d], F32, name="sqs", tag="sqs")
                nc.vector.tensor_mul(sqs, V, V)
                nc.vector.tensor_reduce(
                    s_sq[:, m : m + 1], sqs, mybir.AxisListType.X,
                    mybir.AluOpType.add,
                )

            # ---- t prep: the load and the engine processing are emitted
            # separately so loads can be queued far ahead ----
            g_tiles = {}

            def load_chunk(ch):
                base = ch * chunk
                # grouped layout: partition p holds t rows base+16p..+15,
                # giving 4KB contiguous runs (full DMA bus rate)
                G = chunks.tile(
                    [128, t_per_chunk * d], F32, name="G", tag="G", bufs=8
                )
                nc.sync.dma_start(
                    out=G[:, :].rearrange("p (j d) -> p j d", j=t_per_chunk, d=d),
                    in_=t[base : base + chunk, :].rearrange(
                        "(p j) d -> p j d", p=128, j=t_per_chunk
                    ),
                )
                g_tiles[ch] = G

            def transpose_chunk(ch):
                base = ch * chunk
                G = g_tiles.pop(ch)
                # B columns c = 16p + j: transpose view j, scatter stride 16.
                # All 16 transposes (3-deep PSUM rotation), with the
                # scatter-copy and a square of the just-written B columns
                # (in scatter order, so each square depends only on its own
                # quarter) interleaved on alternating engines.
                Bv = B[0:d, base : base + chunk].rearrange(
                    "e (p j) -> e j p", p=128, j=t_per_chunk
                )
                sqs4 = []
                for j4 in range(t_per_chunk // 4):
                    pst = psum_prep.tile([d, 512], F32, name="pst", tag="pp")
                    for k in range(4):
                        V = G[:, (j4 * 4 + k) * d : (j4 * 4 + k + 1) * d]
                        nc.tensor.transpose(
                            pst[:, k * 128 : (k + 1) * 128], V, identity
                        )
                    dst = Bv[:, j4 * 4 : (j4 + 1) * 4, :]
                    src = pst[:, :].rearrange("e (k p) -> e k p", k=4, p=128)
                    sqv = chunks.tile([d, 512], BF16, name="sq", tag="sq", bufs=4)
                    sqv3 = sqv[:, :].rearrange("e (k p) -> e k p", k=4, p=128)
                    if j4 % 2 == 0:
                        nc.scalar.copy(dst, src)
                        nc.vector.tensor_mul(sqv3, dst, dst)
                    else:
                        nc.vector.tensor_copy(dst, src)
                        nc.scalar.square(sqv3, dst)
                    sqs4.append(sqv)
                return sqs4

            def tsq_chunk(ch, sqs4):
                base = ch * chunk
                # t_sq = ones^T @ sq (bf16, 1 cycle/row), row 64 scattered
                # back with the same (k, p) pattern
                B64v = B[64:65, base : base + chunk].rearrange(
                    "e (p j) -> e j p", p=128, j=t_per_chunk
                )
                for j4, sqv in enumerate(sqs4):
                    pts = psum_pts.tile([128, 512], F32, name="pts", tag="pts")
                    nc.tensor.matmul(pts, ones_mat, sqv, start=True, stop=True)
                    tdst = B64v[:, j4 * 4 : (j4 + 1) * 4, :]
                    tsrc = pts[64:65, :].rearrange("e (k p) -> e k p", k=4, p=128)
                    if j4 % 2 == 0:
                        nc.scalar.copy(tdst, tsrc)
                    else:
                        nc.vector.tensor_copy(tdst, tsrc)

            def process_chunk(ch):
                tsq_chunk(ch, transpose_chunk(ch))

            # ---- main loop tile: one [128, chunk] staging tile.  PSUM
            # tiles span 2 banks (2 matmul windows) so each PSUM->SBUF
            # copy+bias is one [128, 1024] op; the two per tile alternate
            # ACT / DVE. ----
            # output rows of tile m are {16p + m}: affine partition stride
            ov = o[:, :].rearrange("(p m) q -> p m q", p=128, m=m_tiles)

            def main_tile(g, m, split_out=False):
                rows = slice(m * 128, (m + 1) * 128)
                stg = stage.tile([128, chunk], F32, name="stg", tag="stg")
                for ci in range(chunk // 1024):
                    ps = psum_mm.tile([128, 1024], F32, name="ps", tag="mm")
                    for k in range(2):
                        c = (g * chunk) // 512 + ci * 2 + k
                        cols = slice(c * 512, (c + 1) * 512)
                        nc.tensor.matmul(
                            ps[:, k * 512 : (k + 1) * 512], A[:, rows],
                            B[:, cols], start=True, stop=True,
                        )
                    dst = stg[:, ci * 1024 : (ci + 1) * 1024]
                    if (2 * m + ci) % 2 == 0:
                        nc.scalar.add(dst, ps, s_sq[:, m : m + 1])
                    else:
                        nc.vector.tensor_scalar_add(dst, ps, s_sq[:, m : m + 1])
                # alternate the two HWDGE rings (SP / ACT) for 2x the
                # DMA packet-processing throughput on the output stream
                out_eng = nc.sync if m % 2 == 0 else nc.scalar
                if not split_out:
                    out_eng.dma_start(
                        out=ov[:, m, g * chunk : (g + 1) * chunk], in_=stg
                    )
                else:
                    # last tile of the kernel: drain in two halves on both
                    # rings so the tail transfer is half as long
                    h = chunk // 2
                    nc.sync.dma_start(
                        out=ov[:, m, g * chunk : g * chunk + h], in_=stg[:, 0:h]
                    )
                    nc.scalar.dma_start(
                        out=ov[:, m, g * chunk + h : (g + 1) * chunk],
                        in_=stg[:, h:chunk],
                    )

            # software pipeline: the first 4 chunk loads are queued upfront
            # (G has 4 bufs); group g uses chunk g, whose engine processing
            # is emitted right after group g-1's tiles.  Only chunk 0 and
            # the s prep gate the first output DMA.
            # chunk 0 loads first so PE can transpose right after warmup;
            # s (needed later, by the first matmul/adds) follows it.  s uses
            # the same 16-rows-per-partition grouped layout (4KB runs):
            # partition p holds rows 16p..16p+15, so tile m covers rows
            # {16p + m}, and the output DMA addresses them with an affine
            # partition stride of 16 rows.  ALL chunk loads are queued
            # upfront (G has 8 bufs): their ~13us of transfers exactly fill
            # the DMA-idle prep window, so the saturated output stream
            # later never shares the bus with input traffic.
            load_chunk(0)
            nc.sync.dma_start(
                out=S[:, :].rearrange("p (m d) -> p m d", m=m_tiles, d=d),
                in_=s[:, :].rearrange("(p m) d -> p m d", p=128, m=m_tiles),
            )
            for ch in range(1, n_chunks):
                load_chunk(ch)
            # chunk 0: s-prep slots between the transposes and the t_sq
            # matmuls, filling the PE stall while the squares land
            _sq0 = transpose_chunk(0)
            prep_A()
            tsq_chunk(0, _sq0)
            prep_ssq(0)
            prep_ssq(1)
            for g in range(n_chunks):
                for m in range(m_tiles):
                    if g == 0 and m + 2 < m_tiles:
                        prep_ssq(m + 2)
                    # split the first tiles (smoother DMA ramp-up: each
                    # fused add releases its half immediately) and the very
                    # last one (halves the tail transfer)
                    split = (g == 0 and m < 2) or (
                        g == n_chunks - 1 and m == m_tiles - 1
                    )
                    main_tile(g, m, split_out=split)
                if g + 1 < n_chunks:
                    process_chunk(g + 1)

    nc.finalize()
    return nc


_NC_CACHE = {}


def _get_nc(key=None):
    if key is None:
        key = (N_SHARD, Q, D)
    if key not in _NC_CACHE:
        _NC_CACHE[key] = build_nc(*key)
    return _NC_CACHE[key]


def make_in_maps(inputs):
    s = np.asarray(inputs["s"], dtype=np.float32)
    t = np.asarray(inputs["t"], dtype=np.float32)
    assert s.shape == (N, D) and t.shape == (Q, D), (s.shape, t.shape)
    return [{"s": s[c * N_SHARD : (c + 1) * N_SHARD], "t": t} for c in range(N_CORES)]


def _run(inputs, **spmd_kwargs):
    from concourse.bass_utils import run_bass_kernel_spmd

    nc = _get_nc()
    in_maps = make_in_maps(inputs)
    res = run_bass_kernel_spmd(nc, in_maps, list(range(N_CORES)), **spmd_kwargs)
    out = np.concatenate([res.results[c]["o"] for c in range(N_CORES)], axis=0)
    return out, res


def kernel(**inputs):
    out, _ = _run(inputs)
    return out
